# revision 38
# baseline (speedup 1.0000x reference)
"""Dynamic int8-quantized matmul (NPUMatmulLayer) on 8 trn2 NeuronCores.

Reference semantics:
    xq = round(x * 127/max|x|)  (per-tensor, int8 range)
    out = (xq @ W.T) as float32   with W int8 [D_out, D_in]

Sharding: data-parallel over flattened tokens (16384 rows -> 2048/core),
weight replicated. The global abs-max needs one AllReduce(max).

Device kernel (per core):
  - load x^T shard [1024, 2048] f32, DVE absmax-reduce per k-tile
  - AllReduce(max) of per-partition maxes, then GPSIMD partition_all_reduce
  - scale = 127/gmax; quantize: ACT fma(x*s + 1.5*2^23) -> DVE subtract ->
    bf16 integer values (exact, |xq|<=127)
  - bf16 matmul, fp32 PSUM accumulate: exact integer arithmetic since
    |sum| <= 127*32*1024 < 2^24
  - PSUM -> SBUF copies -> DMA out [2048, 1024] f32

x is pre-transposed on the host per shard so the contract dim lands on
SBUF partitions for both matmul operands (no on-device transposes).
"""

import numpy as np
import ml_dtypes

import concourse.bacc as bacc
import concourse.mybir as mybir
import concourse.tile as tile
from concourse import bass_utils, bass_isa

N_CORES = 8
B, S, D_IN, D_OUT = 4, 4096, 1024, 1024
M_TOT = B * S
M_CORE = M_TOT // N_CORES  # 2048
P = 128
KT = D_IN // P  # 8 k-tiles
MAGIC = 12582912.0  # 1.5 * 2**23: f32 round-to-nearest-int via add/sub
F32 = mybir.dt.float32
BF16 = mybir.dt.bfloat16


def build(
    m_core: int = M_CORE,
    n_cores: int = N_CORES,
    repeats: int = 1,
    use_cc: bool = True,
    variant: str = "a",
    cc_mode: str = "ar_opt",
):
    nc = bacc.Bacc("TRN2", target_bir_lowering=False, debug=False, num_devices=n_cores)
    xT = nc.dram_tensor("xT", [D_IN, m_core], F32, kind="ExternalInput")
    wT = nc.dram_tensor("wT", [D_IN, D_OUT], BF16, kind="ExternalInput")
    if variant == "a":
        out = nc.dram_tensor("out", [m_core, D_OUT], F32, kind="ExternalOutput")
        body = _body
    else:
        # variant b: transposed output [D_OUT, m_core]; host un-transposes
        out = nc.dram_tensor("out", [D_OUT, m_core], F32, kind="ExternalOutput")
        body = _body_b

    with tile.TileContext(nc) as tc:
        for _rep in range(repeats):
            body(nc, tc, xT, wT, out, m_core, n_cores, _rep, use_cc, cc_mode)

    nc.compile()
    return nc


def _serialize_rep(nc, scpool, rep, out):
    """For latency benchmarking with repeats>1: returns a zeros [P,1] tile that
    depends on the previous rep's output (None for rep 0). Adding it to each x
    tile gates this rep's pipeline on the previous rep (no numeric effect)."""
    if rep == 0:
        return None
    d = scpool.tile([P, 1], F32, tag="ser_d", name="ser_d")
    nc.sync.dma_start(d[:], out[0:P, 0:1])
    z = scpool.tile([P, 1], F32, tag="ser_z", name="ser_z")
    nc.vector.tensor_scalar_mul(z[:], d[:], 0.0)
    return z


def _absmax_scale(nc, tc, scpool, dram, pmax, n_cores, use_cc, cc_mode="ar"):
    """Combine per-tile partition maxes -> global scale [128,1]."""
    lmax = scpool.tile([P, 1], F32)
    nc.vector.tensor_reduce(
        lmax[:], pmax[:], axis=mybir.AxisListType.X, op=mybir.AluOpType.max
    )
    if n_cores > 1 and use_cc and cc_mode in ("ar", "ar_opt"):
        shared = cc_mode == "ar_opt" and n_cores > 4
        cc_in = dram.tile([P, 1], F32, name="cc_in")
        cc_out = dram.tile(
            [P, 1], F32, name="cc_out",
            addr_space="Shared" if shared else "Local",
        )
        bounce_eng = nc.sync if shared else nc.gpsimd
        bounce_eng.dma_start(cc_in[:], lmax[:])
        nc.gpsimd.collective_compute(
            "AllReduce",
            mybir.AluOpType.max,
            replica_groups=[list(range(n_cores))],
            ins=[cc_in.opt()],
            outs=[cc_out.opt()],
        )
        gpm = scpool.tile([P, 1], F32)
        bounce_eng.dma_start(gpm[:], cc_out[:])
        gmax = scpool.tile([P, 1], F32)
        nc.gpsimd.partition_all_reduce(
            gmax[:], gpm[:], channels=P, reduce_op=bass_isa.ReduceOp.max
        )
    elif n_cores > 1 and use_cc and cc_mode == "ag":
        # local partition-reduce first, AllGather 16 rows/core, reduce again
        lred = scpool.tile([P, 1], F32)
        nc.gpsimd.partition_all_reduce(
            lred[:], lmax[:], channels=P, reduce_op=bass_isa.ReduceOp.max
        )
        rows = P // n_cores
        cc_in = dram.tile([rows, 1], F32)
        cc_out = dram.tile([P, 1], F32)
        nc.gpsimd.dma_start(cc_in[:], lred[0:rows, :])
        nc.gpsimd.collective_compute(
            "AllGather",
            mybir.AluOpType.bypass,
            replica_groups=[list(range(n_cores))],
            ins=[cc_in.opt()],
            outs=[cc_out.opt()],
        )
        gpm = scpool.tile([P, 1], F32)
        nc.gpsimd.dma_start(gpm[:], cc_out[:])
        gmax = scpool.tile([P, 1], F32)
        nc.gpsimd.partition_all_reduce(
            gmax[:], gpm[:], channels=P, reduce_op=bass_isa.ReduceOp.max
        )
    else:
        gmax = scpool.tile([P, 1], F32)
        nc.gpsimd.partition_all_reduce(
            gmax[:], lmax[:], channels=P, reduce_op=bass_isa.ReduceOp.max
        )
    rcp = scpool.tile([P, 1], F32)
    nc.vector.reciprocal(rcp[:], gmax[:])
    scale = scpool.tile([P, 1], F32)
    nc.vector.tensor_scalar_mul(scale[:], rcp[:], 127.0)
    return scale


def _body_b(nc, tc, xT, wT, out, m_core, n_cores, rep, use_cc=True, cc_mode="ar"):
    """W^T-stationary variant: 64 LDWEIGHTS total, output transposed
    [D_OUT, m_core] (host fixes layout at gather)."""
    with (
        tc.tile_pool(name=f"x_{rep}", bufs=1) as xpool,
        tc.tile_pool(name=f"xq_{rep}", bufs=1) as xqpool,
        tc.tile_pool(name=f"w_{rep}", bufs=1) as wpool,
        tc.tile_pool(name=f"sc_{rep}", bufs=1) as scpool,
        tc.tile_pool(name=f"t_{rep}", bufs=3) as tpool,
        tc.tile_pool(name=f"ob_{rep}", bufs=3) as opool,
        tc.tile_pool(name=f"ps_{rep}", bufs=2, space="PSUM") as pspool,
        tc.tile_pool(name=f"dram_{rep}", bufs=1, space="DRAM") as dram,
    ):
        ser_z = _serialize_rep(nc, scpool, rep, out)
        pmax = scpool.tile([P, KT], F32)
        xts = []
        for k in range(KT):
            xt = xpool.tile([P, m_core], F32, tag=f"x{k}")
            if ser_z is not None:
                nc.vector.tensor_copy(xt[:, 0:1], ser_z[:])
            nc.sync.dma_start(xt[:], xT[P * k : P * (k + 1), :])
            nc.vector.tensor_reduce(
                pmax[:, k : k + 1],
                xt[:],
                axis=mybir.AxisListType.X,
                op=mybir.AluOpType.max,
                apply_absolute_value=True,
            )
            xts.append(xt)
        wts = []
        for k in range(KT):
            wt = wpool.tile([P, D_OUT], BF16, tag=f"w{k}")
            if ser_z is not None:
                nc.vector.tensor_copy(wt[:, 0:1], ser_z[:])
            nc.sync.dma_start(wt[:], wT[P * k : P * (k + 1), :])
            wts.append(wt)

        scale = _absmax_scale(nc, tc, scpool, dram, pmax, n_cores, use_cc, cc_mode)

        # quantize full k-tiles (fewer, larger ACT/DVE ops)
        xqs = []
        for k in range(KT):
            xq = xqpool.tile([P, m_core], BF16, tag=f"xq{k}", name=f"xq{k}")
            t = tpool.tile([P, m_core], F32, tag="t", name="t")
            nc.scalar.activation(
                t[:], xts[k][:], mybir.ActivationFunctionType.Copy, scale=scale[:]
            )
            nc.vector.tensor_scalar(
                xq[:], t[:], MAGIC, -MAGIC,
                op0=mybir.AluOpType.add, op1=mybir.AluOpType.add,
            )
            xqs.append(xq)

        mc_cols = min(512, m_core)
        n_mc = m_core // mc_cols
        for ot in range(D_OUT // P):  # 8 output tiles
            osl = slice(ot * P, (ot + 1) * P)
            pss = [
                pspool.tile([P, mc_cols], F32, tag=f"ps{mc}", name=f"ps{mc}")
                for mc in range(n_mc)
            ]
            for k in range(KT):
                for mc in range(n_mc):
                    nc.tensor.matmul(
                        pss[mc][:],
                        wts[k][:, osl],
                        xqs[k][:, mc * mc_cols : (mc + 1) * mc_cols],
                        start=(k == 0),
                        stop=(k == KT - 1),
                    )
            ob = opool.tile([P, m_core], F32)
            for mc in range(n_mc):
                msl = slice(mc * mc_cols, (mc + 1) * mc_cols)
                if mc % 2 == 0:
                    nc.scalar.copy(ob[:, msl], pss[mc][:])
                else:
                    nc.vector.tensor_copy(ob[:, msl], pss[mc][:])
            nc.sync.dma_start(out[osl, :], ob[:])


def _launch_ar(nc, scpool, dram, pmax_slice, i, n_cores):
    """Reduce a pmax column range and launch an AllReduce(max) on it.
    Returns the SBUF tile that will hold the collective result."""
    lm = scpool.tile([P, 1], F32, tag=f"lm{i}", name=f"lm{i}")
    nc.vector.tensor_reduce(
        lm[:], pmax_slice, axis=mybir.AxisListType.X, op=mybir.AluOpType.max
    )
    cc_in = dram.tile([P, 1], F32, name=f"cc_in{i}")
    cc_out = dram.tile(
        [P, 1], F32, name=f"cc_out{i}",
        addr_space="Shared" if n_cores > 4 else "Local",
    )
    nc.sync.dma_start(cc_in[:], lm[:])
    nc.gpsimd.collective_compute(
        "AllReduce",
        mybir.AluOpType.max,
        replica_groups=[list(range(n_cores))],
        ins=[cc_in.opt()],
        outs=[cc_out.opt()],
    )
    o = scpool.tile([P, 1], F32, tag=f"aro{i}", name=f"aro{i}")
    nc.sync.dma_start(o[:], cc_out[:])
    return o


def _body(nc, tc, xT, wT, out, m_core, n_cores, rep, use_cc=True, cc_mode="ar"):
    with (
            tc.tile_pool(name=f"x_{rep}", bufs=1) as xpool,
            tc.tile_pool(name=f"xq_{rep}", bufs=1) as xqpool,
            tc.tile_pool(name=f"w_{rep}", bufs=1) as wpool,
            tc.tile_pool(name=f"sc_{rep}", bufs=1) as scpool,
            tc.tile_pool(name=f"t_{rep}", bufs=3) as tpool,
            tc.tile_pool(name=f"ob_{rep}", bufs=3) as opool,
            tc.tile_pool(name=f"ps_{rep}", bufs=4, space="PSUM") as pspool,
            tc.tile_pool(name=f"dram_{rep}", bufs=1, space="DRAM") as dram,
        ):
            # ---- load x shard + per-partition absmax ----
            # loads/reduces chunked in halves: shortens the serial reduce
            # tail after the last chunk lands
            ser_z = _serialize_rep(nc, scpool, rep, out)
            halves = 2 if m_core % 2 == 0 else 1
            half = m_core // halves
            pmax = scpool.tile([P, KT * halves], F32)
            split_ar = n_cores > 1 and use_cc and cc_mode == "ar2"
            ar_outs = []
            xts = []
            for k in range(KT):
                xt = xpool.tile([P, m_core], F32, tag=f"x{k}")
                for h in range(halves):
                    hsl = slice(h * half, (h + 1) * half)
                    if ser_z is not None:
                        # WAW gate: the load itself waits for the previous rep
                        nc.vector.tensor_copy(
                            xt[:, h * half : h * half + 1], ser_z[:]
                        )
                    nc.sync.dma_start(xt[:, hsl], xT[P * k : P * (k + 1), hsl])
                    nc.vector.tensor_reduce(
                        pmax[:, k * halves + h : k * halves + h + 1],
                        xt[:, hsl],
                        axis=mybir.AxisListType.X,
                        op=mybir.AluOpType.max,
                        apply_absolute_value=True,
                    )
                xts.append(xt)
                if split_ar and k == KT // 2 - 1:
                    # AR#1 on the first half of x, launched mid-load
                    ncols = (KT // 2) * halves
                    ar_outs.append(
                        _launch_ar(nc, scpool, dram, pmax[:, 0:ncols], 0, n_cores)
                    )
            if split_ar:
                ncols = (KT // 2) * halves
                ar_outs.append(
                    _launch_ar(
                        nc, scpool, dram, pmax[:, ncols : KT * halves], 1, n_cores
                    )
                )

            # ---- replicated weight (already transposed+bf16 on host) ----
            wts = []
            for k in range(KT):
                wt = wpool.tile([P, D_OUT], BF16, tag=f"w{k}")
                if ser_z is not None:
                    nc.vector.tensor_copy(
                        wt[:, 0:1], ser_z[:]
                    )  # WAW gate (dtype cast ok)
                nc.sync.dma_start(wt[:], wT[P * k : P * (k + 1), :])
                wts.append(wt)

            if split_ar:
                gpm = scpool.tile([P, 1], F32)
                nc.vector.tensor_max(gpm[:], ar_outs[0][:], ar_outs[1][:])
                gmax = scpool.tile([P, 1], F32)
                nc.gpsimd.partition_all_reduce(
                    gmax[:], gpm[:], channels=P, reduce_op=bass_isa.ReduceOp.max
                )
                rcp = scpool.tile([P, 1], F32)
                nc.vector.reciprocal(rcp[:], gmax[:])
                scale = scpool.tile([P, 1], F32)
                nc.vector.tensor_scalar_mul(scale[:], rcp[:], 127.0)
            else:
                scale = _absmax_scale(
                    nc, tc, scpool, dram, pmax, n_cores, use_cc, cc_mode
                )

            # ---- quantize (by m-group) + matmul + store ----
            xqs = [
                xqpool.tile([P, m_core], BF16, tag=f"xq{k}", name=f"xq{k}")
                for k in range(KT)
            ]
            gcols = min(512, m_core)
            for g in range(m_core // gcols):
                sl = slice(g * gcols, (g + 1) * gcols)
                for k in range(KT):
                    t = tpool.tile([P, gcols], F32)
                    # t = x*scale (single f32 rounding, matches reference mul)
                    nc.scalar.activation(
                        t[:],
                        xts[k][:, sl],
                        mybir.ActivationFunctionType.Copy,
                        scale=scale[:],
                    )
                    # round-to-nearest-even via +MAGIC, -MAGIC (each add
                    # rounds in f32), cast bf16 (exact: integers <= 127)
                    nc.vector.tensor_scalar(
                        xqs[k][:, sl],
                        t[:],
                        MAGIC,
                        -MAGIC,
                        op0=mybir.AluOpType.add,
                        op1=mybir.AluOpType.add,
                    )
                for mi in range(gcols // P):
                    m = g * (gcols // P) + mi
                    msl = slice(m * P, (m + 1) * P)
                    ps0 = pspool.tile([P, 512], F32, tag="ps0")
                    ps1 = pspool.tile([P, 512], F32, tag="ps1")
                    for k in range(KT):
                        nc.tensor.matmul(
                            ps0[:],
                            xqs[k][:, msl],
                            wts[k][:, 0:512],
                            start=(k == 0),
                            stop=(k == KT - 1),
                        )
                        nc.tensor.matmul(
                            ps1[:],
                            xqs[k][:, msl],
                            wts[k][:, 512:1024],
                            start=(k == 0),
                            stop=(k == KT - 1),
                        )
                    ob = opool.tile([P, D_OUT], F32)
                    nc.scalar.copy(ob[:, 0:512], ps0[:])
                    nc.vector.tensor_copy(ob[:, 512:1024], ps1[:])
                    nc.sync.dma_start(out[msl, :], ob[:])


VARIANT = "a"

_nc_cache: dict = {}


def _get_nc(m_core: int = M_CORE, n_cores: int = N_CORES, variant: str | None = None):
    if variant is None:
        variant = VARIANT
    key = (m_core, n_cores, variant)
    if key not in _nc_cache:
        _nc_cache[key] = build(m_core, n_cores, variant=variant)
    return _nc_cache[key]


def make_in_maps(x: np.ndarray, weight: np.ndarray, n_cores: int = N_CORES):
    xf = np.asarray(x, dtype=np.float32).reshape(-1, D_IN)
    m_core = xf.shape[0] // n_cores
    w_bf16 = np.ascontiguousarray(
        np.asarray(weight, dtype=np.float32).T
    ).astype(ml_dtypes.bfloat16)
    in_maps = []
    for c in range(n_cores):
        shard = xf[c * m_core : (c + 1) * m_core]  # [m_core, D_IN]
        xT_c = np.ascontiguousarray(shard.T)  # [D_IN, m_core]
        in_maps.append({"xT": xT_c, "wT": w_bf16})
    return in_maps, m_core


def gather_out(res, variant: str | None = None):
    if variant is None:
        variant = VARIANT
    if variant == "a":
        shards = [res.results[c]["out"] for c in range(N_CORES)]
    else:
        shards = [res.results[c]["out"].T for c in range(N_CORES)]
    out = np.concatenate(shards, axis=0)
    return np.ascontiguousarray(out).reshape(B, S, D_OUT).astype(np.float32)


def kernel(x: np.ndarray, weight: np.ndarray) -> np.ndarray:
    in_maps, m_core = make_in_maps(x, weight)
    nc = _get_nc(m_core)
    res = bass_utils.run_bass_kernel_spmd(
        nc, in_maps, core_ids=list(range(N_CORES))
    )
    return gather_out(res)


# revision 39
# speedup vs baseline: 1.1299x; 1.1299x over previous
"""Dynamic int8-quantized matmul (NPUMatmulLayer) on 8 trn2 NeuronCores.

Reference semantics:
    xq = round(x * 127/max|x|)  (per-tensor, int8 range)
    out = (xq @ W.T) as float32   with W int8 [D_out, D_in]

Sharding: data-parallel over flattened tokens (16384 rows -> 2048/core),
weight replicated. The global abs-max needs one AllReduce(max).

Device kernel (per core):
  - load x^T shard [1024, 2048] f32, DVE absmax-reduce per k-tile
  - AllReduce(max) of per-partition maxes, then GPSIMD partition_all_reduce
  - scale = 127/gmax; quantize: ACT fma(x*s + 1.5*2^23) -> DVE subtract ->
    bf16 integer values (exact, |xq|<=127)
  - bf16 matmul, fp32 PSUM accumulate: exact integer arithmetic since
    |sum| <= 127*32*1024 < 2^24
  - PSUM -> SBUF copies -> DMA out [2048, 1024] f32

x is pre-transposed on the host per shard so the contract dim lands on
SBUF partitions for both matmul operands (no on-device transposes).
"""

import numpy as np
import ml_dtypes

import concourse.bacc as bacc
import concourse.mybir as mybir
import concourse.tile as tile
from concourse import bass_utils, bass_isa

N_CORES = 8
B, S, D_IN, D_OUT = 4, 4096, 1024, 1024
M_TOT = B * S
M_CORE = M_TOT // N_CORES  # 2048
P = 128
KT = D_IN // P  # 8 k-tiles
MAGIC = 12582912.0  # 1.5 * 2**23: f32 round-to-nearest-int via add/sub
F32 = mybir.dt.float32
BF16 = mybir.dt.bfloat16


def build(
    m_core: int = M_CORE,
    n_cores: int = N_CORES,
    repeats: int = 1,
    use_cc: bool = True,
    variant: str = "a",
    cc_mode: str = "ar_opt",
):
    nc = bacc.Bacc("TRN2", target_bir_lowering=False, debug=False, num_devices=n_cores)
    xT = nc.dram_tensor("xT", [D_IN, m_core], F32, kind="ExternalInput")
    wT = nc.dram_tensor("wT", [D_IN, D_OUT], BF16, kind="ExternalInput")
    if variant == "a":
        out = nc.dram_tensor("out", [m_core, D_OUT], F32, kind="ExternalOutput")
        body = _body
    else:
        # variant b: transposed output [D_OUT, m_core]; host un-transposes
        out = nc.dram_tensor("out", [D_OUT, m_core], F32, kind="ExternalOutput")
        body = _body_b

    with tile.TileContext(nc) as tc:
        for _rep in range(repeats):
            body(nc, tc, xT, wT, out, m_core, n_cores, _rep, use_cc, cc_mode)

    nc.compile()
    return nc


def _serialize_rep(nc, scpool, rep, out):
    """For latency benchmarking with repeats>1: returns a zeros [P,1] tile that
    depends on the previous rep's output (None for rep 0). Adding it to each x
    tile gates this rep's pipeline on the previous rep (no numeric effect)."""
    if rep == 0:
        return None
    d = scpool.tile([P, 1], F32, tag="ser_d", name="ser_d")
    nc.sync.dma_start(d[:], out[0:P, 0:1])
    z = scpool.tile([P, 1], F32, tag="ser_z", name="ser_z")
    nc.vector.tensor_scalar_mul(z[:], d[:], 0.0)
    return z


def _absmax_scale(nc, tc, scpool, dram, pmax, n_cores, use_cc, cc_mode="ar"):
    """Combine per-tile partition maxes -> global scale [128,1]."""
    lmax = scpool.tile([P, 1], F32)
    nc.vector.tensor_reduce(
        lmax[:], pmax[:], axis=mybir.AxisListType.X, op=mybir.AluOpType.max
    )
    if n_cores > 1 and use_cc and cc_mode in ("ar", "ar_opt"):
        shared = cc_mode == "ar_opt" and n_cores > 4
        cc_in = dram.tile([P, 1], F32, name="cc_in")
        cc_out = dram.tile(
            [P, 1], F32, name="cc_out",
            addr_space="Shared" if shared else "Local",
        )
        bounce_eng = nc.sync if shared else nc.gpsimd
        bounce_eng.dma_start(cc_in[:], lmax[:])
        nc.gpsimd.collective_compute(
            "AllReduce",
            mybir.AluOpType.max,
            replica_groups=[list(range(n_cores))],
            ins=[cc_in.opt()],
            outs=[cc_out.opt()],
        )
        gpm = scpool.tile([P, 1], F32)
        bounce_eng.dma_start(gpm[:], cc_out[:])
        gmax = scpool.tile([P, 1], F32)
        nc.gpsimd.partition_all_reduce(
            gmax[:], gpm[:], channels=P, reduce_op=bass_isa.ReduceOp.max
        )
    elif n_cores > 1 and use_cc and cc_mode == "ag":
        # local partition-reduce first, AllGather 16 rows/core, reduce again
        lred = scpool.tile([P, 1], F32)
        nc.gpsimd.partition_all_reduce(
            lred[:], lmax[:], channels=P, reduce_op=bass_isa.ReduceOp.max
        )
        rows = P // n_cores
        cc_in = dram.tile([rows, 1], F32)
        cc_out = dram.tile([P, 1], F32)
        nc.gpsimd.dma_start(cc_in[:], lred[0:rows, :])
        nc.gpsimd.collective_compute(
            "AllGather",
            mybir.AluOpType.bypass,
            replica_groups=[list(range(n_cores))],
            ins=[cc_in.opt()],
            outs=[cc_out.opt()],
        )
        gpm = scpool.tile([P, 1], F32)
        nc.gpsimd.dma_start(gpm[:], cc_out[:])
        gmax = scpool.tile([P, 1], F32)
        nc.gpsimd.partition_all_reduce(
            gmax[:], gpm[:], channels=P, reduce_op=bass_isa.ReduceOp.max
        )
    else:
        gmax = scpool.tile([P, 1], F32)
        nc.gpsimd.partition_all_reduce(
            gmax[:], lmax[:], channels=P, reduce_op=bass_isa.ReduceOp.max
        )
    rcp = scpool.tile([P, 1], F32)
    nc.vector.reciprocal(rcp[:], gmax[:])
    scale = scpool.tile([P, 1], F32)
    nc.vector.tensor_scalar_mul(scale[:], rcp[:], 127.0)
    return scale


def _body_b(nc, tc, xT, wT, out, m_core, n_cores, rep, use_cc=True, cc_mode="ar"):
    """W^T-stationary variant: 64 LDWEIGHTS total, output transposed
    [D_OUT, m_core] (host fixes layout at gather)."""
    with (
        tc.tile_pool(name=f"x_{rep}", bufs=1) as xpool,
        tc.tile_pool(name=f"xq_{rep}", bufs=1) as xqpool,
        tc.tile_pool(name=f"w_{rep}", bufs=1) as wpool,
        tc.tile_pool(name=f"sc_{rep}", bufs=1) as scpool,
        tc.tile_pool(name=f"t_{rep}", bufs=3) as tpool,
        tc.tile_pool(name=f"ob_{rep}", bufs=3) as opool,
        tc.tile_pool(name=f"ps_{rep}", bufs=2, space="PSUM") as pspool,
        tc.tile_pool(name=f"dram_{rep}", bufs=1, space="DRAM") as dram,
    ):
        ser_z = _serialize_rep(nc, scpool, rep, out)
        pmax = scpool.tile([P, KT], F32)
        xts = []
        for k in range(KT):
            xt = xpool.tile([P, m_core], F32, tag=f"x{k}")
            if ser_z is not None:
                nc.vector.tensor_copy(xt[:, 0:1], ser_z[:])
            nc.sync.dma_start(xt[:], xT[P * k : P * (k + 1), :])
            nc.vector.tensor_reduce(
                pmax[:, k : k + 1],
                xt[:],
                axis=mybir.AxisListType.X,
                op=mybir.AluOpType.max,
                apply_absolute_value=True,
            )
            xts.append(xt)
        wts = []
        for k in range(KT):
            wt = wpool.tile([P, D_OUT], BF16, tag=f"w{k}")
            if ser_z is not None:
                nc.vector.tensor_copy(wt[:, 0:1], ser_z[:])
            nc.sync.dma_start(wt[:], wT[P * k : P * (k + 1), :])
            wts.append(wt)

        scale = _absmax_scale(nc, tc, scpool, dram, pmax, n_cores, use_cc, cc_mode)

        # quantize full k-tiles (fewer, larger ACT/DVE ops)
        xqs = []
        for k in range(KT):
            xq = xqpool.tile([P, m_core], BF16, tag=f"xq{k}", name=f"xq{k}")
            t = tpool.tile([P, m_core], F32, tag="t", name="t")
            nc.scalar.activation(
                t[:], xts[k][:], mybir.ActivationFunctionType.Copy, scale=scale[:]
            )
            nc.vector.tensor_scalar(
                xq[:], t[:], MAGIC, -MAGIC,
                op0=mybir.AluOpType.add, op1=mybir.AluOpType.add,
            )
            xqs.append(xq)

        mc_cols = min(512, m_core)
        n_mc = m_core // mc_cols
        for ot in range(D_OUT // P):  # 8 output tiles
            osl = slice(ot * P, (ot + 1) * P)
            pss = [
                pspool.tile([P, mc_cols], F32, tag=f"ps{mc}", name=f"ps{mc}")
                for mc in range(n_mc)
            ]
            for k in range(KT):
                for mc in range(n_mc):
                    nc.tensor.matmul(
                        pss[mc][:],
                        wts[k][:, osl],
                        xqs[k][:, mc * mc_cols : (mc + 1) * mc_cols],
                        start=(k == 0),
                        stop=(k == KT - 1),
                    )
            ob = opool.tile([P, m_core], F32)
            for mc in range(n_mc):
                msl = slice(mc * mc_cols, (mc + 1) * mc_cols)
                if mc % 2 == 0:
                    nc.scalar.copy(ob[:, msl], pss[mc][:])
                else:
                    nc.vector.tensor_copy(ob[:, msl], pss[mc][:])
            nc.sync.dma_start(out[osl, :], ob[:])


def _launch_ar(nc, scpool, dram, pmax_slice, i, n_cores):
    """Reduce a pmax column range and launch an AllReduce(max) on it.
    Returns the SBUF tile that will hold the collective result."""
    lm = scpool.tile([P, 1], F32, tag=f"lm{i}", name=f"lm{i}")
    nc.vector.tensor_reduce(
        lm[:], pmax_slice, axis=mybir.AxisListType.X, op=mybir.AluOpType.max
    )
    cc_in = dram.tile([P, 1], F32, name=f"cc_in{i}")
    cc_out = dram.tile(
        [P, 1], F32, name=f"cc_out{i}",
        addr_space="Shared" if n_cores > 4 else "Local",
    )
    nc.sync.dma_start(cc_in[:], lm[:])
    nc.gpsimd.collective_compute(
        "AllReduce",
        mybir.AluOpType.max,
        replica_groups=[list(range(n_cores))],
        ins=[cc_in.opt()],
        outs=[cc_out.opt()],
    )
    o = scpool.tile([P, 1], F32, tag=f"aro{i}", name=f"aro{i}")
    nc.sync.dma_start(o[:], cc_out[:])
    return o


def _body(nc, tc, xT, wT, out, m_core, n_cores, rep, use_cc=True, cc_mode="ar"):
    with (
            tc.tile_pool(name=f"x_{rep}", bufs=1) as xpool,
            tc.tile_pool(name=f"xq_{rep}", bufs=1) as xqpool,
            tc.tile_pool(name=f"w_{rep}", bufs=1) as wpool,
            tc.tile_pool(name=f"sc_{rep}", bufs=1) as scpool,
            tc.tile_pool(name=f"t_{rep}", bufs=3) as tpool,
            tc.tile_pool(name=f"ob_{rep}", bufs=3) as opool,
            tc.tile_pool(name=f"ps_{rep}", bufs=4, space="PSUM") as pspool,
            tc.tile_pool(name=f"dram_{rep}", bufs=1, space="DRAM") as dram,
        ):
            # ---- load x shard + per-partition absmax ----
            # loads/reduces chunked in halves: shortens the serial reduce
            # tail after the last chunk lands
            ser_z = _serialize_rep(nc, scpool, rep, out)
            halves = 4 if m_core % 4 == 0 else 1
            half = m_core // halves
            pmax = scpool.tile([P, KT * halves], F32)
            split_ar = n_cores > 1 and use_cc and cc_mode == "ar2"
            ar_outs = []
            xts = []
            for k in range(KT):
                xt = xpool.tile([P, m_core], F32, tag=f"x{k}")
                for h in range(halves):
                    hsl = slice(h * half, (h + 1) * half)
                    if ser_z is not None:
                        # WAW gate: the load itself waits for the previous rep
                        nc.vector.tensor_copy(
                            xt[:, h * half : h * half + 1], ser_z[:]
                        )
                    nc.sync.dma_start(xt[:, hsl], xT[P * k : P * (k + 1), hsl])
                    nc.vector.tensor_reduce(
                        pmax[:, k * halves + h : k * halves + h + 1],
                        xt[:, hsl],
                        axis=mybir.AxisListType.X,
                        op=mybir.AluOpType.max,
                        apply_absolute_value=True,
                    )
                xts.append(xt)
                if split_ar and k == KT // 2 - 1:
                    # AR#1 on the first half of x, launched mid-load
                    ncols = (KT // 2) * halves
                    ar_outs.append(
                        _launch_ar(nc, scpool, dram, pmax[:, 0:ncols], 0, n_cores)
                    )
            if split_ar:
                ncols = (KT // 2) * halves
                ar_outs.append(
                    _launch_ar(
                        nc, scpool, dram, pmax[:, ncols : KT * halves], 1, n_cores
                    )
                )

            # ---- replicated weight (already transposed+bf16 on host) ----
            wts = []
            for k in range(KT):
                wt = wpool.tile([P, D_OUT], BF16, tag=f"w{k}")
                if ser_z is not None:
                    nc.vector.tensor_copy(
                        wt[:, 0:1], ser_z[:]
                    )  # WAW gate (dtype cast ok)
                nc.sync.dma_start(wt[:], wT[P * k : P * (k + 1), :])
                wts.append(wt)

            if split_ar:
                gpm = scpool.tile([P, 1], F32)
                nc.vector.tensor_max(gpm[:], ar_outs[0][:], ar_outs[1][:])
                gmax = scpool.tile([P, 1], F32)
                nc.gpsimd.partition_all_reduce(
                    gmax[:], gpm[:], channels=P, reduce_op=bass_isa.ReduceOp.max
                )
                rcp = scpool.tile([P, 1], F32)
                nc.vector.reciprocal(rcp[:], gmax[:])
                scale = scpool.tile([P, 1], F32)
                nc.vector.tensor_scalar_mul(scale[:], rcp[:], 127.0)
            else:
                scale = _absmax_scale(
                    nc, tc, scpool, dram, pmax, n_cores, use_cc, cc_mode
                )

            # ---- quantize (by m-group) + matmul + store ----
            xqs = [
                xqpool.tile([P, m_core], BF16, tag=f"xq{k}", name=f"xq{k}")
                for k in range(KT)
            ]
            gcols = min(512, m_core)
            for g in range(m_core // gcols):
                sl = slice(g * gcols, (g + 1) * gcols)
                for k in range(KT):
                    t = tpool.tile([P, gcols], F32)
                    # t = x*scale (single f32 rounding, matches reference mul)
                    nc.scalar.activation(
                        t[:],
                        xts[k][:, sl],
                        mybir.ActivationFunctionType.Copy,
                        scale=scale[:],
                    )
                    # round-to-nearest-even via +MAGIC, -MAGIC (each add
                    # rounds in f32), cast bf16 (exact: integers <= 127)
                    nc.vector.tensor_scalar(
                        xqs[k][:, sl],
                        t[:],
                        MAGIC,
                        -MAGIC,
                        op0=mybir.AluOpType.add,
                        op1=mybir.AluOpType.add,
                    )
                for mi in range(gcols // P):
                    m = g * (gcols // P) + mi
                    msl = slice(m * P, (m + 1) * P)
                    ps0 = pspool.tile([P, 512], F32, tag="ps0")
                    ps1 = pspool.tile([P, 512], F32, tag="ps1")
                    for k in range(KT):
                        nc.tensor.matmul(
                            ps0[:],
                            xqs[k][:, msl],
                            wts[k][:, 0:512],
                            start=(k == 0),
                            stop=(k == KT - 1),
                        )
                        nc.tensor.matmul(
                            ps1[:],
                            xqs[k][:, msl],
                            wts[k][:, 512:1024],
                            start=(k == 0),
                            stop=(k == KT - 1),
                        )
                    ob = opool.tile([P, D_OUT], F32)
                    nc.scalar.copy(ob[:, 0:512], ps0[:])
                    nc.vector.tensor_copy(ob[:, 512:1024], ps1[:])
                    nc.sync.dma_start(out[msl, :], ob[:])


VARIANT = "a"

_nc_cache: dict = {}


def _get_nc(m_core: int = M_CORE, n_cores: int = N_CORES, variant: str | None = None):
    if variant is None:
        variant = VARIANT
    key = (m_core, n_cores, variant)
    if key not in _nc_cache:
        _nc_cache[key] = build(m_core, n_cores, variant=variant)
    return _nc_cache[key]


def make_in_maps(x: np.ndarray, weight: np.ndarray, n_cores: int = N_CORES):
    xf = np.asarray(x, dtype=np.float32).reshape(-1, D_IN)
    m_core = xf.shape[0] // n_cores
    w_bf16 = np.ascontiguousarray(
        np.asarray(weight, dtype=np.float32).T
    ).astype(ml_dtypes.bfloat16)
    in_maps = []
    for c in range(n_cores):
        shard = xf[c * m_core : (c + 1) * m_core]  # [m_core, D_IN]
        xT_c = np.ascontiguousarray(shard.T)  # [D_IN, m_core]
        in_maps.append({"xT": xT_c, "wT": w_bf16})
    return in_maps, m_core


def gather_out(res, variant: str | None = None):
    if variant is None:
        variant = VARIANT
    if variant == "a":
        shards = [res.results[c]["out"] for c in range(N_CORES)]
    else:
        shards = [res.results[c]["out"].T for c in range(N_CORES)]
    out = np.concatenate(shards, axis=0)
    return np.ascontiguousarray(out).reshape(B, S, D_OUT).astype(np.float32)


def kernel(x: np.ndarray, weight: np.ndarray) -> np.ndarray:
    in_maps, m_core = make_in_maps(x, weight)
    nc = _get_nc(m_core)
    res = bass_utils.run_bass_kernel_spmd(
        nc, in_maps, core_ids=list(range(N_CORES))
    )
    return gather_out(res)
